# revision 6
# baseline (speedup 1.0000x reference)
"""Weighted Kabsch/Procrustes (B=16, N=200000) on 8 Trainium2 NeuronCores.

Sharding: data-parallel over the batch dim — each of the 8 cores handles 2
batches independently (no cross-core communication).

Device kernel (per core, per batch):
  All 16 reductions needed by weighted Kabsch are computed on-chip:
    M_raw[c,d] = sum_i w_i * x_c(i) * y_d(i)   (9)
    my[d]      = sum_i w_i * y_d(i)            (3)
    mx[c]      = sum_i w_i * x_c(i)            (3)
    S          = sum_i w_i                     (1)
  The 12 sums {M_raw, my} come from a block-diagonal TensorEngine trick:
  the stationary operand packs 32 point-chunks as [w*x0, w*x1, w*x2, w]
  column-quads; the moving operand is the raw interleaved ref tile (3 cols
  per chunk).  Diagonal 4x3 blocks of the PSUM accumulator are the wanted
  sums; off-diagonal blocks are garbage that the host ignores.  mx and S
  use fused DVE tensor_tensor_reduce / ScalarE accumulate.
  Host does the remaining O(B) work: centroid algebra, 3x3 SVD, R, t.

Layout: per batch, the 200000 points are split as 125 partitions x 1600
points; point (p, t) = flat index p*1600 + t, so every DMA moves long
contiguous runs per partition.
"""

import numpy as np

# ---- hardcoded problem geometry (from spec) ----
B_TOTAL = 16
N_PTS = 200000
N_CORES = 8
NB = B_TOTAL // N_CORES  # batches per core = 2

P = 125            # SBUF partitions used (125*1600 = 200000)
TPB = 1600         # points per partition per batch
CH = 5             # chunks per batch
TC = TPB // CH     # 320 points (t-positions) per chunk
JG = 32            # point-chunks (t-positions) per matmul group
G = TC // JG       # 10 matmul groups per chunk
EPS = 1e-5

_NC_CACHE = {}


def _build_nc():
    from contextlib import ExitStack
    import concourse.bacc as bacc
    import concourse.tile as tile
    import concourse.mybir as mybir

    f32 = mybir.dt.float32
    mult = mybir.AluOpType.mult
    add = mybir.AluOpType.add

    nc = bacc.Bacc("TRN2", target_bir_lowering=False, debug=False)

    src = nc.dram_tensor("src", [NB, N_PTS, 3], f32, kind="ExternalInput")
    ref = nc.dram_tensor("ref", [NB, N_PTS, 3], f32, kind="ExternalInput")
    wts = nc.dram_tensor("wts", [NB, N_PTS], f32, kind="ExternalInput")
    out_mm = nc.dram_tensor("out_mm", [NB, 128, 3 * JG], f32, kind="ExternalOutput")
    out_aux = nc.dram_tensor("out_aux", [NB, P, 3 * CH + CH], f32, kind="ExternalOutput")

    srcv = src[:].rearrange("b (p f) c -> b p (f c)", p=P)  # [NB, 125, 4800]
    refv = ref[:].rearrange("b (p f) c -> b p (f c)", p=P)
    wv = wts[:].rearrange("b (p f) -> b p f", p=P)          # [NB, 125, 1600]

    with tile.TileContext(nc) as tc, ExitStack() as ctx:
        loads = ctx.enter_context(tc.tile_pool(name="loads", bufs=3))
        wxp = ctx.enter_context(tc.tile_pool(name="wxp", bufs=3))
        scr = ctx.enter_context(tc.tile_pool(name="scr", bufs=2))
        accs = ctx.enter_context(tc.tile_pool(name="accs", bufs=2))
        psum = ctx.enter_context(tc.tile_pool(name="psum", bufs=2, space="PSUM"))
        outp = ctx.enter_context(tc.tile_pool(name="outp", bufs=2))

        for b in range(NB):
            mxacc = accs.tile([P, 3 * CH], f32, tag="mxacc")
            sacc = accs.tile([P, CH], f32, tag="sacc")
            pmm = psum.tile([128, 3 * JG], f32, tag="pmm")

            for c in range(CH):
                xc = loads.tile([P, 3 * TC], f32, tag="xc")
                yc = loads.tile([P, 3 * TC], f32, tag="yc")
                wc = loads.tile([P, TC], f32, tag="wc")
                nc.sync.dma_start(out=xc, in_=srcv[b, :, c * 3 * TC:(c + 1) * 3 * TC])
                nc.sync.dma_start(out=yc, in_=refv[b, :, c * 3 * TC:(c + 1) * 3 * TC])
                nc.sync.dma_start(out=wc, in_=wv[b, :, c * TC:(c + 1) * TC])

                # stationary operand: [w*x0, w*x1, w*x2, w] per point
                wxw = wxp.tile([P, 4 * TC], f32, tag="wxw")
                x3 = xc.rearrange("p (t c) -> p t c", c=3)
                w4 = wxw.rearrange("p (t f) -> p t f", f=4)
                for a in range(3):
                    nc.vector.tensor_mul(w4[:, :, a], x3[:, :, a], wc)
                nc.scalar.copy(out=w4[:, :, 3], in_=wc)

                # S = sum(w): ScalarE pass-through with accumulate
                sc2 = scr.tile([P, TC], f32, tag="scr2")
                nc.scalar.activation(
                    out=sc2,
                    in_=wc,
                    func=mybir.ActivationFunctionType.Copy,
                    accum_out=sacc[:, c:c + 1],
                )
                # mx_a = sum(w * x_a): ScalarE re-reads the wxw columns
                for a in range(3):
                    sca = scr.tile([P, TC], f32, tag="scra")
                    nc.scalar.activation(
                        out=sca,
                        in_=w4[:, :, a],
                        func=mybir.ActivationFunctionType.Copy,
                        accum_out=mxacc[:, c * 3 + a:c * 3 + a + 1],
                    )

                for g in range(G):
                    nc.tensor.matmul(
                        out=pmm,
                        lhsT=wxw[:, g * 4 * JG:(g + 1) * 4 * JG],
                        rhs=yc[:, g * 3 * JG:(g + 1) * 3 * JG],
                        start=(c == 0 and g == 0),
                        stop=(c == CH - 1 and g == G - 1),
                    )

            mm_st = outp.tile([128, 3 * JG], f32, tag="mmst")
            nc.scalar.copy(out=mm_st, in_=pmm)
            nc.sync.dma_start(out=out_mm[b], in_=mm_st)
            nc.sync.dma_start(out=out_aux[b, :, 0:3 * CH], in_=mxacc)
            nc.sync.dma_start(out=out_aux[b, :, 3 * CH:], in_=sacc)

    nc.compile()
    return nc


def get_nc():
    if "nc" not in _NC_CACHE:
        _NC_CACHE["nc"] = _build_nc()
    return _NC_CACHE["nc"]


def _postprocess(out_mm, out_aux):
    """From the device sums of one batch, compute (R, t) on host (float64)."""
    blocks = out_mm.reshape(JG, 4, JG, 3)
    diag = blocks[np.arange(JG), :, np.arange(JG), :]        # [32, 4, 3]
    sums = diag.sum(axis=0, dtype=np.float64)                # [4, 3]
    M_raw = sums[:3, :]                                      # sum w x_c y_d
    my = sums[3, :]                                          # sum w y_d
    mx = out_aux[:, :3 * CH].reshape(P, CH, 3).sum(axis=(0, 1), dtype=np.float64)
    S = out_aux[:, 3 * CH:].sum(dtype=np.float64)

    denom = S + EPS
    xbar = mx / denom                                        # (3,)
    ybar = my / denom
    s = S / denom
    M = M_raw / denom
    H = M - (2.0 - s) * np.outer(xbar, ybar)

    U, _, Vh = np.linalg.svd(H)
    V = Vh.T
    d = np.sign(np.linalg.det(V @ U.T))
    if d == 0:
        d = 1.0
    D = np.diag([1.0, 1.0, d])
    R = V @ D @ U.T
    t = ybar - R @ xbar
    return R, t


def kernel(src_points, ref_points, weights):
    import os
    # The axon NTFF profile hook (antenv.axon_hooks) is absent in this
    # container; a trace request would crash run_bass_kernel_spmd.
    os.environ["BASS_NEVER_TRACE"] = "1"
    from concourse.bass_utils import run_bass_kernel_spmd

    src = np.ascontiguousarray(np.asarray(src_points, dtype=np.float32))
    ref = np.ascontiguousarray(np.asarray(ref_points, dtype=np.float32))
    w = np.ascontiguousarray(np.asarray(weights, dtype=np.float32))
    assert src.shape == (B_TOTAL, N_PTS, 3), src.shape
    assert ref.shape == (B_TOTAL, N_PTS, 3), ref.shape
    assert w.shape == (B_TOTAL, N_PTS), w.shape

    nc = get_nc()
    in_maps = [
        {
            "src": src[i * NB:(i + 1) * NB],
            "ref": ref[i * NB:(i + 1) * NB],
            "wts": w[i * NB:(i + 1) * NB],
        }
        for i in range(N_CORES)
    ]
    res = run_bass_kernel_spmd(nc, in_maps, core_ids=list(range(N_CORES)))
    _NC_CACHE["last_result"] = res

    R = np.zeros((B_TOTAL, 3, 3), dtype=np.float32)
    t = np.zeros((B_TOTAL, 3), dtype=np.float32)
    for i, core_res in enumerate(res.results):
        for b in range(NB):
            Rb, tb = _postprocess(core_res["out_mm"][b], core_res["out_aux"][b])
            R[i * NB + b] = Rb.astype(np.float32)
            t[i * NB + b] = tb.astype(np.float32)
    return R, t


# revision 7
# speedup vs baseline: 1.0740x; 1.0740x over previous
"""Weighted Kabsch/Procrustes (B=16, N=200000) on 8 Trainium2 NeuronCores.

Sharding: data-parallel over the batch dim — each of the 8 cores handles 2
batches independently (no cross-core communication).

Device kernel (per core, per batch):
  All 16 reductions needed by weighted Kabsch are computed on-chip:
    M_raw[c,d] = sum_i w_i * x_c(i) * y_d(i)   (9)
    my[d]      = sum_i w_i * y_d(i)            (3)
    mx[c]      = sum_i w_i * x_c(i)            (3)
    S          = sum_i w_i                     (1)
  The 12 sums {M_raw, my} come from a block-diagonal TensorEngine trick:
  the stationary operand packs 32 point-chunks as [w*x0, w*x1, w*x2, w]
  column-quads; the moving operand is the raw interleaved ref tile (3 cols
  per chunk).  Diagonal 4x3 blocks of the PSUM accumulator are the wanted
  sums; off-diagonal blocks are garbage that the host ignores.  mx and S
  use fused DVE tensor_tensor_reduce / ScalarE accumulate.
  Host does the remaining O(B) work: centroid algebra, 3x3 SVD, R, t.

Layout: per batch, the 200000 points are split as 125 partitions x 1600
points; point (p, t) = flat index p*1600 + t, so every DMA moves long
contiguous runs per partition.
"""

import numpy as np

# ---- hardcoded problem geometry (from spec) ----
B_TOTAL = 16
N_PTS = 200000
N_CORES = 8
NB = B_TOTAL // N_CORES  # batches per core = 2

P = 125            # SBUF partitions used (125*1600 = 200000)
TPB = 1600         # points per partition per batch
CH = 5             # chunks per batch
TC = TPB // CH     # 320 points (t-positions) per chunk
JG = 32            # point-chunks (t-positions) per matmul group
G = TC // JG       # 10 matmul groups per chunk
EPS = 1e-5

_NC_CACHE = {}


def _build_nc():
    from contextlib import ExitStack
    import concourse.bacc as bacc
    import concourse.tile as tile
    import concourse.mybir as mybir

    f32 = mybir.dt.float32
    mult = mybir.AluOpType.mult
    add = mybir.AluOpType.add

    nc = bacc.Bacc("TRN2", target_bir_lowering=False, debug=False)

    src = nc.dram_tensor("src", [NB, N_PTS, 3], f32, kind="ExternalInput")
    ref = nc.dram_tensor("ref", [NB, N_PTS, 3], f32, kind="ExternalInput")
    wts = nc.dram_tensor("wts", [NB, N_PTS], f32, kind="ExternalInput")
    out_mm = nc.dram_tensor("out_mm", [NB, 128, 3 * JG], f32, kind="ExternalOutput")
    out_aux = nc.dram_tensor("out_aux", [NB, P, 3 * CH + CH], f32, kind="ExternalOutput")

    srcv = src[:].rearrange("b (p f) c -> b p (f c)", p=P)  # [NB, 125, 4800]
    refv = ref[:].rearrange("b (p f) c -> b p (f c)", p=P)
    wv = wts[:].rearrange("b (p f) -> b p f", p=P)          # [NB, 125, 1600]

    with tile.TileContext(nc) as tc, ExitStack() as ctx:
        loads = ctx.enter_context(tc.tile_pool(name="loads", bufs=4))
        wxp = ctx.enter_context(tc.tile_pool(name="wxp", bufs=4))
        scr = ctx.enter_context(tc.tile_pool(name="scr", bufs=2))
        accs = ctx.enter_context(tc.tile_pool(name="accs", bufs=2))
        psum = ctx.enter_context(tc.tile_pool(name="psum", bufs=2, space="PSUM"))
        outp = ctx.enter_context(tc.tile_pool(name="outp", bufs=2))

        for b in range(NB):
            mxacc = accs.tile([P, 3 * CH], f32, tag="mxacc")
            sacc = accs.tile([P, CH], f32, tag="sacc")
            pmm = psum.tile([128, 3 * JG], f32, tag="pmm")

            for c in range(CH):
                xc = loads.tile([P, 3 * TC], f32, tag="xc")
                yc = loads.tile([P, 3 * TC], f32, tag="yc")
                wc = loads.tile([P, TC], f32, tag="wc")
                nc.sync.dma_start(out=xc, in_=srcv[b, :, c * 3 * TC:(c + 1) * 3 * TC])
                nc.sync.dma_start(out=yc, in_=refv[b, :, c * 3 * TC:(c + 1) * 3 * TC])
                nc.sync.dma_start(out=wc, in_=wv[b, :, c * TC:(c + 1) * TC])

                # stationary operand: [w*x0, w*x1, w*x2, w] per point
                wxw = wxp.tile([P, 4 * TC], f32, tag="wxw")
                x3 = xc.rearrange("p (t c) -> p t c", c=3)
                w4 = wxw.rearrange("p (t f) -> p t f", f=4)
                for a in range(3):
                    nc.vector.tensor_mul(w4[:, :, a], x3[:, :, a], wc)
                nc.scalar.copy(out=w4[:, :, 3], in_=wc)

                # S = sum(w): ScalarE pass-through with accumulate
                sc2 = scr.tile([P, TC], f32, tag="scr2")
                nc.scalar.activation(
                    out=sc2,
                    in_=wc,
                    func=mybir.ActivationFunctionType.Copy,
                    accum_out=sacc[:, c:c + 1],
                )
                # mx_a = sum(w * x_a): ScalarE re-reads the wxw columns
                for a in range(3):
                    sca = scr.tile([P, TC], f32, tag="scra")
                    nc.scalar.activation(
                        out=sca,
                        in_=w4[:, :, a],
                        func=mybir.ActivationFunctionType.Copy,
                        accum_out=mxacc[:, c * 3 + a:c * 3 + a + 1],
                    )

                for g in range(G):
                    nc.tensor.matmul(
                        out=pmm,
                        lhsT=wxw[:, g * 4 * JG:(g + 1) * 4 * JG],
                        rhs=yc[:, g * 3 * JG:(g + 1) * 3 * JG],
                        start=(c == 0 and g == 0),
                        stop=(c == CH - 1 and g == G - 1),
                    )

            mm_st = outp.tile([128, 3 * JG], f32, tag="mmst")
            nc.scalar.copy(out=mm_st, in_=pmm)
            nc.sync.dma_start(out=out_mm[b], in_=mm_st)
            nc.sync.dma_start(out=out_aux[b, :, 0:3 * CH], in_=mxacc)
            nc.sync.dma_start(out=out_aux[b, :, 3 * CH:], in_=sacc)

    nc.compile()
    return nc


def get_nc():
    if "nc" not in _NC_CACHE:
        _NC_CACHE["nc"] = _build_nc()
    return _NC_CACHE["nc"]


def _postprocess(out_mm, out_aux):
    """From the device sums of one batch, compute (R, t) on host (float64)."""
    blocks = out_mm.reshape(JG, 4, JG, 3)
    diag = blocks[np.arange(JG), :, np.arange(JG), :]        # [32, 4, 3]
    sums = diag.sum(axis=0, dtype=np.float64)                # [4, 3]
    M_raw = sums[:3, :]                                      # sum w x_c y_d
    my = sums[3, :]                                          # sum w y_d
    mx = out_aux[:, :3 * CH].reshape(P, CH, 3).sum(axis=(0, 1), dtype=np.float64)
    S = out_aux[:, 3 * CH:].sum(dtype=np.float64)

    denom = S + EPS
    xbar = mx / denom                                        # (3,)
    ybar = my / denom
    s = S / denom
    M = M_raw / denom
    H = M - (2.0 - s) * np.outer(xbar, ybar)

    U, _, Vh = np.linalg.svd(H)
    V = Vh.T
    d = np.sign(np.linalg.det(V @ U.T))
    if d == 0:
        d = 1.0
    D = np.diag([1.0, 1.0, d])
    R = V @ D @ U.T
    t = ybar - R @ xbar
    return R, t


def kernel(src_points, ref_points, weights):
    import os
    # The axon NTFF profile hook (antenv.axon_hooks) is absent in this
    # container; a trace request would crash run_bass_kernel_spmd.
    os.environ["BASS_NEVER_TRACE"] = "1"
    from concourse.bass_utils import run_bass_kernel_spmd

    src = np.ascontiguousarray(np.asarray(src_points, dtype=np.float32))
    ref = np.ascontiguousarray(np.asarray(ref_points, dtype=np.float32))
    w = np.ascontiguousarray(np.asarray(weights, dtype=np.float32))
    assert src.shape == (B_TOTAL, N_PTS, 3), src.shape
    assert ref.shape == (B_TOTAL, N_PTS, 3), ref.shape
    assert w.shape == (B_TOTAL, N_PTS), w.shape

    nc = get_nc()
    in_maps = [
        {
            "src": src[i * NB:(i + 1) * NB],
            "ref": ref[i * NB:(i + 1) * NB],
            "wts": w[i * NB:(i + 1) * NB],
        }
        for i in range(N_CORES)
    ]
    res = run_bass_kernel_spmd(nc, in_maps, core_ids=list(range(N_CORES)))
    _NC_CACHE["last_result"] = res

    R = np.zeros((B_TOTAL, 3, 3), dtype=np.float32)
    t = np.zeros((B_TOTAL, 3), dtype=np.float32)
    for i, core_res in enumerate(res.results):
        for b in range(NB):
            Rb, tb = _postprocess(core_res["out_mm"][b], core_res["out_aux"][b])
            R[i * NB + b] = Rb.astype(np.float32)
            t[i * NB + b] = tb.astype(np.float32)
    return R, t
